# revision 20
# baseline (speedup 1.0000x reference)
"""MLA (DeepSeek-style) attention layer on 8 Trainium2 NeuronCores.

Sharding: core c -> batch b = c//4, head group g = c%4 (4 of 16 heads).
Each core computes a partial output (its heads' contribution through its
W_O row-slice); the host sums the 4 partials per batch.

Device layouts are feature-major ("transposed"): activations stored as
[feature, seq] so the PE contraction dim (partitions) is the feature dim.
The q/k path runs in fp32r (full-rate, ~1.6e-4 matmul precision) to keep
the attention logits accurate; probabilities/values/W_O run in bf16.
The probability matrix is transposed back via DMA-xbar (bf16) for the AV
matmul, and the final W_O matmul un-transposes into natural [seq, D].
RMSNorm weights are folded into the up-projection weights on the host;
the inverse-rms per-token scale is applied on PSUM eviction.

The kernel is mask-adaptive: the host classifies the mask input as
"zero" (all zeros -> dense attention, no mask applied), "causal"
(tril zeros / upper <= -1e8 -> block-causal skipping: QK chunks, softmax
stats, exp, transposes and AV tiles above the diagonal are skipped, and
the diagonal 512-key chunk gets one of 4 precomputed 128x512 triangle
patterns), or "general" (full additive-mask path). One program per
variant is built and cached.
"""
import sys

for _p in ("/opt/trn_rl_repo", "/root/.axon_site/_ro/trn_rl_repo"):
    if _p not in sys.path:
        sys.path.append(_p)

import numpy as np
import ml_dtypes

B, S, D = 2, 2048, 2048
H, NOPE, ROPE, VD = 16, 128, 64, 128
DCQ, DCKV = 1536, 512
EPS = 1e-6
SCALE = float(np.sqrt(NOPE + ROPE))
HL = 4           # local heads per core
NCORES = 8
NQT = S // 128   # 16
NKC = S // 512   # 4
NKD = D // 128   # 16
NMQ = DCQ // 128  # 12
NMKV = DCKV // 128  # 4
BF = ml_dtypes.bfloat16

_BUILD_CACHE = {}


def build_nc(variant="causal", taps=False):
    import concourse.tile as tile
    import concourse.mybir as mybir
    from concourse import bacc

    F32 = mybir.dt.float32
    F32R = mybir.dt.float32r
    BF16 = mybir.dt.bfloat16

    nc = bacc.Bacc(num_devices=NCORES)

    T = {}
    T["xT"] = nc.dram_tensor("xT", [D, S], F32R, kind="ExternalInput")
    if variant == "general":
        T["maskp"] = nc.dram_tensor("maskp", [S, S], BF16, kind="ExternalInput")
    elif variant == "causal":
        T["dmask"] = nc.dram_tensor("dmask", [128, 2048], BF16, kind="ExternalInput")
    T["cos4"] = nc.dram_tensor("cos4", [128, S], F32, kind="ExternalInput")
    T["sin4"] = nc.dram_tensor("sin4", [128, S], F32, kind="ExternalInput")
    T["wdq"] = nc.dram_tensor("wdq", [D, DCQ], F32R, kind="ExternalInput")
    T["wdkv"] = nc.dram_tensor("wdkv", [D, DCKV], F32R, kind="ExternalInput")
    T["wdkr"] = nc.dram_tensor("wdkr", [D, 64], F32R, kind="ExternalInput")
    T["wuq"] = nc.dram_tensor("wuq", [DCQ, HL * NOPE], F32R, kind="ExternalInput")
    T["wuqre"] = nc.dram_tensor("wuqre", [DCQ, HL * 32], F32R, kind="ExternalInput")
    T["wuqro"] = nc.dram_tensor("wuqro", [DCQ, HL * 32], F32R, kind="ExternalInput")
    T["wuk"] = nc.dram_tensor("wuk", [DCKV, HL * NOPE], F32R, kind="ExternalInput")
    T["wuv"] = nc.dram_tensor("wuv", [DCKV, HL * VD], F32R, kind="ExternalInput")
    T["wo4"] = nc.dram_tensor("wo4", [HL * VD, D], BF16, kind="ExternalInput")
    T["ident"] = nc.dram_tensor("ident", [128, 128], BF16, kind="ExternalInput")
    T["ones_r"] = nc.dram_tensor("ones_r", [1, 128], F32R, kind="ExternalInput")
    T["ones_c"] = nc.dram_tensor("ones_c", [128, 1], F32R, kind="ExternalInput")
    T["outp"] = nc.dram_tensor("outp", [S, D], F32, kind="ExternalOutput")
    if taps:
        T["tap_cq"] = nc.dram_tensor("tap_cq", [DCQ, S], F32R, kind="ExternalOutput")
        T["tap_inv"] = nc.dram_tensor("tap_inv", [2, S], F32, kind="ExternalOutput")
        T["tap_q"] = nc.dram_tensor("tap_q", [HL * NOPE, S], F32R, kind="ExternalOutput")
        T["tap_k"] = nc.dram_tensor("tap_k", [HL * NOPE, S], F32R, kind="ExternalOutput")
        T["tap_qr"] = nc.dram_tensor("tap_qr", [256, S], F32R, kind="ExternalOutput")
        T["tap_kr"] = nc.dram_tensor("tap_kr", [128, S], F32R, kind="ExternalOutput")
        T["tap_v"] = nc.dram_tensor("tap_v", [NQT * 128, HL * VD], BF16, kind="ExternalOutput")
        T["tap_p"] = nc.dram_tensor("tap_p", [512, S], BF16, kind="ExternalOutput")

    with tile.TileContext(nc) as tc:
        _emit(nc, tc, T, variant, taps)
    nc.compile()
    return nc


def _emit(nc, tc, T, variant, taps):
    import concourse.bass as bass
    import concourse.mybir as mybir

    F32 = mybir.dt.float32
    F32R = mybir.dt.float32r
    BF16 = mybir.dt.bfloat16
    AF = mybir.ActivationFunctionType
    AX = mybir.AxisListType
    ts = bass.ts

    xT, cos4, sin4 = T["xT"], T["cos4"], T["sin4"]
    wdq, wdkv, wdkr = T["wdq"], T["wdkv"], T["wdkr"]
    wuq, wuqre, wuqro, wuk, wuv, wo4 = (
        T["wuq"], T["wuqre"], T["wuqro"], T["wuk"], T["wuv"], T["wo4"])
    ident, ones_r, ones_c, outp = T["ident"], T["ones_r"], T["ones_c"], T["outp"]

    # --- persistent-scope pools, opened in lifetime (LIFO) order ---
    const_p = tc.tile_pool(name="constp", bufs=1)
    const = const_p.__enter__()
    onesr_t = const.tile([1, 128], F32R, tag="onesr")
    nc.sync.dma_start(onesr_t[:], ones_r[:])
    onesc_t = const.tile([128, 1], F32R, tag="onesc")
    nc.sync.dma_start(onesc_t[:], ones_c[:])
    ident_t = const.tile([128, 128], BF16, tag="ident")
    nc.sync.dma_start(ident_t[:], ident[:])

    dram_p = tc.tile_pool(name="dram", bufs=1, space="DRAM")
    dram = dram_p.__enter__()
    cqd = dram.tile([NMQ, 128, S], F32R, tag="cqd")
    ckvd = dram.tile([NMKV, 128, S], F32R, tag="ckvd")

    kfeat_p = tc.tile_pool(name="kfeat", bufs=1)
    kfeat = kfeat_p.__enter__()
    krope2 = kfeat.tile([128, S], F32R, tag="krope2")

    rows_p = tc.tile_pool(name="rowsp", bufs=1)
    rows = rows_p.__enter__()
    sum_rows = rows.tile([33, S], F32, tag="sum_rows")
    invq_row = rows.tile([1, S], F32R, tag="invq_row")
    invkv_row = rows.tile([1, S], F32R, tag="invkv_row")

    # ============ Phase A: down-projection (c spilled to DRAM) + k-rope ====
    # W_DQ (12 MiB) + W_DKR stay resident across all S-quarters; W_DKV
    # streams per quarter (SBUF won't hold all three plus x).
    with tc.tile_pool(name="wdown", bufs=1) as wdp, \
         tc.tile_pool(name="wkvA", bufs=2) as wkvp, \
         tc.tile_pool(name="xA", bufs=1) as xpool, \
         tc.tile_pool(name="evA", bufs=2) as evpool, \
         tc.tile_pool(name="sqA", bufs=2) as sqpool, \
         tc.tile_pool(name="ropeA", bufs=1) as ropeA, \
         tc.tile_pool(name="psA", bufs=3, space="PSUM") as psA, \
         tc.tile_pool(name="psSum", bufs=2, space="PSUM") as psSum:
        wd_tiles = []
        for m in range(NMQ):
            t = wdp.tile([128, NKD, 128], F32R, tag=f"wd{m}")
            nc.sync.dma_start(
                t[:],
                wdq.rearrange("(kt p) m -> p kt m", p=128)[:, :, m * 128:(m + 1) * 128])
            wd_tiles.append(t)
        wkr_t = wdp.tile([128, NKD, 64], F32R, tag="wkr")
        nc.sync.dma_start(wkr_t[:], wdkr.rearrange("(kt p) m -> p kt m", p=128))
        for quar in range(4):
            sl = slice(quar * 512, (quar + 1) * 512)
            xq = []
            for k in range(NKD):
                t = xpool.tile([128, 512], F32R, tag=f"xq{k}", name=f"xq{quar}_{k}")
                nc.sync.dma_start(t[:], xT[ts(k, 128), sl])
                xq.append(t)
            sum_ps_q = psSum.tile([1, 512], F32, tag="sumq")
            sum_ps_kv = psSum.tile([1, 512], F32, tag="sumkv")
            for m in range(NMQ + NMKV):
                if m < NMQ:
                    wm = wd_tiles[m]
                else:
                    wm = wkvp.tile([128, NKD, 128], F32R, tag="wkv",
                                   name=f"wkv{quar}_{m - NMQ}")
                    nc.sync.dma_start(
                        wm[:],
                        wdkv.rearrange("(kt p) m -> p kt m", p=128)
                        [:, :, (m - NMQ) * 128:(m - NMQ + 1) * 128])
                ps = psA.tile([128, 512], F32, tag="dp", name=f"dp{quar}_{m}")
                for k in range(NKD):
                    nc.tensor.matmul(ps[:], wm[:, k, :], xq[k][:],
                                     start=(k == 0), stop=(k == NKD - 1))
                ev = evpool.tile([128, 512], F32R, tag="cev", name=f"cev{quar}_{m}")
                nc.scalar.activation(ev[:], ps[:], AF.Copy)
                dst = cqd[m] if m < NMQ else ckvd[m - NMQ]
                nc.sync.dma_start(dst[:, sl], ev[:])
                sq = sqpool.tile([128, 512], F32R, tag="sq", name=f"sq{quar}_{m}")
                nc.scalar.activation(sq[:], ps[:], AF.Square)
                if m < NMQ:
                    nc.tensor.matmul(sum_ps_q[:], onesc_t[:], sq[:],
                                     start=(m == 0), stop=(m == NMQ - 1))
                else:
                    nc.tensor.matmul(sum_ps_kv[:], onesc_t[:], sq[:],
                                     start=(m == NMQ), stop=(m == NMQ + NMKV - 1))
            # merged 64-col W_DKR tile (even rope dims in 0:32, odd in 32:64)
            psr = psA.tile([128, 512], F32, tag="dp", name=f"dpr{quar}")
            for k in range(NKD):
                nc.tensor.matmul(psr[:64, :], wkr_t[:, k, :], xq[k][:],
                                 start=(k == 0), stop=(k == NKD - 1))
            kre_q = ropeA.tile([32, 512], F32, tag="kr0", name=f"kre{quar}")
            nc.scalar.activation(kre_q[:], psr[0:32, :], AF.Copy)
            kro_q = ropeA.tile([32, 512], F32, tag="kr1", name=f"kro{quar}")
            nc.scalar.activation(kro_q[:], psr[32:64, :], AF.Copy)
            # k-side rope for this quarter (k_R has no rms norm)
            cs_a = ropeA.tile([32, 512], F32, tag="cs_a", name=f"cs{quar}")
            nc.sync.dma_start(cs_a[:], cos4[0:32, sl])
            sn_a = ropeA.tile([32, 512], F32, tag="sn_a", name=f"sn{quar}")
            nc.sync.dma_start(sn_a[:], sin4[0:32, sl])
            t1k = ropeA.tile([32, 512], F32, tag="t1k", bufs=1, name=f"t1k{quar}")
            nc.vector.tensor_mul(t1k[:], kre_q[:], cs_a[:])
            t2k = ropeA.tile([32, 512], F32, tag="t2k", bufs=1, name=f"t2k{quar}")
            nc.vector.tensor_mul(t2k[:], kro_q[:], sn_a[:])
            ko1 = ropeA.tile([32, 512], F32R, tag="ko1", name=f"ko1{quar}")
            nc.vector.tensor_sub(ko1[:], t1k[:], t2k[:])
            t3k = ropeA.tile([32, 512], F32, tag="t1k", bufs=1, name=f"t3k{quar}")
            nc.vector.tensor_mul(t3k[:], kre_q[:], sn_a[:])
            t4k = ropeA.tile([32, 512], F32, tag="t2k", bufs=1, name=f"t4k{quar}")
            nc.vector.tensor_mul(t4k[:], kro_q[:], cs_a[:])
            ko2 = ropeA.tile([32, 512], F32R, tag="ko2", name=f"ko2{quar}")
            nc.vector.tensor_add(ko2[:], t3k[:], t4k[:])
            for rep in range(2):
                nc.sync.dma_start(krope2[ts(rep * 2, 32), sl], ko1[:])
                nc.sync.dma_start(krope2[ts(rep * 2 + 1, 32), sl], ko2[:])
            nc.vector.tensor_copy(sum_rows[0:1, sl], sum_ps_q[:])
            nc.vector.tensor_copy(sum_rows[32:33, sl], sum_ps_kv[:])

    # ---- inverse rms rows (chunk broadcasts built on demand in B1/B2) ----
    with tc.tile_pool(name="invp", bufs=1) as invp:
        epst = invp.tile([1, 1], F32, tag="epst")
        nc.gpsimd.memset(epst[:], EPS)
        rms2 = invp.tile([33, S], F32, tag="rms2")
        nc.scalar.activation(rms2[0:1, :], sum_rows[0:1, :], AF.Sqrt,
                             bias=epst[:], scale=1.0 / DCQ)
        nc.scalar.activation(rms2[32:33, :], sum_rows[32:33, :], AF.Sqrt,
                             bias=epst[:], scale=1.0 / DCKV)
        with nc.allow_low_precision(reason="f32r shares f32 bits"):
            nc.vector.reciprocal(invq_row[:], rms2[0:1, :])
            nc.vector.reciprocal(invkv_row[:], rms2[32:33, :])
    if taps:
        nc.sync.dma_start(T["tap_inv"][0:1, :], invq_row[:])
        nc.sync.dma_start(T["tap_inv"][1:2, :], invkv_row[:])
        for m in range(NMQ):
            nc.sync.dma_start(T["tap_cq"][ts(m, 128), :], cqd[m])

    # ============ Phase B1: q-side up-projection + rope ============
    qside_p = tc.tile_pool(name="qside", bufs=1)
    qside = qside_p.__enter__()
    qT = [qside.tile([128, S], F32R, tag=f"qT{h}", name=f"qT{h}") for h in range(HL)]
    qrope = [qside.tile([128, S], F32R, tag=f"qrope{p}", name=f"qrope{p}")
             for p in range(2)]

    with tc.tile_pool(name="wB1", bufs=1) as wb1, \
         tc.tile_pool(name="csB1", bufs=2) as csB1, \
         tc.tile_pool(name="cqs", bufs=1) as cqs, \
         tc.tile_pool(name="bcB1", bufs=2) as bcB1, \
         tc.tile_pool(name="ropeS", bufs=2) as ropeS, \
         tc.tile_pool(name="psB", bufs=3, space="PSUM") as psB:
        wuq_t = wb1.tile([128, NMQ, HL * NOPE], F32R, tag="wuq")
        nc.sync.dma_start(wuq_t[:], wuq.rearrange("(kt p) m -> p kt m", p=128))
        wuqre_t = wb1.tile([128, NMQ, HL * 32], F32R, tag="wuqre")
        nc.sync.dma_start(wuqre_t[:], wuqre.rearrange("(kt p) m -> p kt m", p=128))
        wuqro_t = wb1.tile([128, NMQ, HL * 32], F32R, tag="wuqro")
        nc.sync.dma_start(wuqro_t[:], wuqro.rearrange("(kt p) m -> p kt m", p=128))
        for n in range(NKC):
            sl = slice(n * 512, (n + 1) * 512)
            cos_t = csB1.tile([128, 512], F32, tag="cos", name=f"cos{n}")
            nc.sync.dma_start(cos_t[:], cos4[:, sl])
            sin_t = csB1.tile([128, 512], F32, tag="sin", name=f"sin{n}")
            nc.sync.dma_start(sin_t[:], sin4[:, sl])
            cq = []
            for k in range(NMQ):
                t = cqs.tile([128, 512], F32R, tag=f"cqc{k}", name=f"cqc{n}_{k}")
                nc.sync.dma_start(t[:], cqd[k][:, sl])
                cq.append(t)
            psbc = psB.tile([128, 512], F32, tag="bc", bufs=2, name=f"bcq{n}")
            nc.tensor.matmul(psbc[:], onesr_t[:], invq_row[0:1, sl],
                             start=True, stop=True)
            bcq_t = bcB1.tile([128, 512], F32, tag="bcq", name=f"bcqs{n}")
            nc.scalar.activation(bcq_t[:], psbc[:], AF.Copy)
            for h in range(HL):
                ps = psB.tile([128, 512], F32, tag="up", name=f"upq{n}_{h}")
                for k in range(NMQ):
                    nc.tensor.matmul(ps[:], wuq_t[:, k, ts(h, 128)], cq[k][:],
                                     start=(k == 0), stop=(k == NMQ - 1))
                nc.vector.tensor_mul(qT[h][:, sl], ps[:], bcq_t[:])
            psE = psB.tile([128, 512], F32, tag="up", name=f"upe{n}")
            for k in range(NMQ):
                nc.tensor.matmul(psE[:], wuqre_t[:, k, :], cq[k][:],
                                 start=(k == 0), stop=(k == NMQ - 1))
            esc = ropeS.tile([128, 512], F32, tag="esc", bufs=1, name=f"esc{n}")
            nc.vector.tensor_mul(esc[:], psE[:], bcq_t[:])
            psO = psB.tile([128, 512], F32, tag="up", name=f"upo{n}")
            for k in range(NMQ):
                nc.tensor.matmul(psO[:], wuqro_t[:, k, :], cq[k][:],
                                 start=(k == 0), stop=(k == NMQ - 1))
            osc = ropeS.tile([128, 512], F32, tag="osc", bufs=1, name=f"osc{n}")
            nc.vector.tensor_mul(osc[:], psO[:], bcq_t[:])
            t1 = ropeS.tile([128, 512], F32, tag="t1", bufs=1, name=f"t1{n}")
            nc.vector.tensor_mul(t1[:], esc[:], cos_t[:])
            t2 = ropeS.tile([128, 512], F32, tag="t2", bufs=1, name=f"t2{n}")
            nc.vector.tensor_mul(t2[:], osc[:], sin_t[:])
            o1 = ropeS.tile([128, 512], F32R, tag="o1", name=f"o1{n}")
            nc.vector.tensor_sub(o1[:], t1[:], t2[:])
            t3 = ropeS.tile([128, 512], F32, tag="t1", bufs=1, name=f"t3{n}")
            nc.vector.tensor_mul(t3[:], esc[:], sin_t[:])
            t4 = ropeS.tile([128, 512], F32, tag="t2", bufs=1, name=f"t4{n}")
            nc.vector.tensor_mul(t4[:], osc[:], cos_t[:])
            o2 = ropeS.tile([128, 512], F32R, tag="o2", name=f"o2{n}")
            nc.vector.tensor_add(o2[:], t3[:], t4[:])
            for h in range(HL):
                p, off = h // 2, (h % 2) * 64
                nc.sync.dma_start(qrope[p][off:off + 32, sl], o1[ts(h, 32), :])
                nc.sync.dma_start(qrope[p][off + 32:off + 64, sl], o2[ts(h, 32), :])

    if taps:
        for h in range(HL):
            nc.sync.dma_start(T["tap_q"][ts(h, 128), :], qT[h][:])
        nc.sync.dma_start(T["tap_qr"][0:128, :], qrope[0][:])
        nc.sync.dma_start(T["tap_qr"][128:256, :], qrope[1][:])
        nc.sync.dma_start(T["tap_kr"][:], krope2[:])

    # ============ Phase B2: kv-side up-projection ============
    kside_p = tc.tile_pool(name="kside", bufs=1)
    kside = kside_p.__enter__()
    kT = [kside.tile([128, S], F32R, tag=f"kT{h}", name=f"kT{h}") for h in range(HL)]
    v_all = kside.tile([128, NQT, HL * VD], BF16, tag="v_all")

    with tc.tile_pool(name="wB2", bufs=1) as wb2, \
         tc.tile_pool(name="ckvs", bufs=2) as ckvs, \
         tc.tile_pool(name="psB2", bufs=3, space="PSUM") as psB2:
        wuk_t = wb2.tile([128, NMKV, HL * NOPE], F32R, tag="wuk")
        nc.sync.dma_start(wuk_t[:], wuk.rearrange("(kt p) m -> p kt m", p=128))
        wuv_t = wb2.tile([128, NMKV, HL * VD], F32R, tag="wuv")
        nc.sync.dma_start(wuv_t[:], wuv.rearrange("(kt p) m -> p kt m", p=128))
        for n in range(NKC):
            sl = slice(n * 512, (n + 1) * 512)
            psbc = psB2.tile([128, 512], F32, tag="bc", bufs=2, name=f"bckv{n}")
            nc.tensor.matmul(psbc[:], onesr_t[:], invkv_row[0:1, sl],
                             start=True, stop=True)
            bckv_t = ckvs.tile([128, 512], F32, tag="bckv", name=f"bckvs{n}")
            nc.scalar.activation(bckv_t[:], psbc[:], AF.Copy)
            ckv = []
            for k in range(NMKV):
                t = ckvs.tile([128, 512], F32R, tag=f"ckvc{k}", name=f"ckvc{n}_{k}")
                nc.sync.dma_start(t[:], ckvd[k][:, sl])
                tn = ckvs.tile([128, 512], F32R, tag=f"ckvn{k}", name=f"ckvn{n}_{k}")
                nc.vector.tensor_mul(tn[:], t[:], bckv_t[:])
                ckv.append(tn)
            for h in range(HL):
                ps = psB2.tile([128, 512], F32, tag="upk", name=f"upk{n}_{h}")
                for k in range(NMKV):
                    nc.tensor.matmul(ps[:], wuk_t[:, k, ts(h, 128)], ckv[k][:],
                                     start=(k == 0), stop=(k == NMKV - 1))
                nc.scalar.activation(kT[h][:, sl], ps[:], AF.Copy)
            for vm in range(4):
                m = n * 4 + vm
                ps = psB2.tile([128, 512], F32, tag="upv", name=f"upv{n}_{vm}")
                for k in range(NMKV):
                    nc.tensor.matmul(ps[:], ckv[k][:, ts(vm, 128)], wuv_t[:, k, :],
                                     start=(k == 0), stop=(k == NMKV - 1))
                nc.scalar.activation(v_all[:, m, :], ps[:], AF.Copy)
    if taps:
        for h in range(HL):
            nc.sync.dma_start(T["tap_k"][ts(h, 128), :], kT[h][:])
        for m in range(NQT):
            nc.sync.dma_start(T["tap_v"][ts(m, 128), :], v_all[:, m, :])

    # ============ Attention ============
    with tc.tile_pool(name="wo", bufs=1) as wop, \
         tc.tile_pool(name="maskP", bufs=1) as maskpl, \
         tc.tile_pool(name="pu", bufs=2) as pup, \
         tc.tile_pool(name="pn", bufs=2) as pnp, \
         tc.tile_pool(name="pT", bufs=2) as pTp, \
         tc.tile_pool(name="attP", bufs=1) as attp, \
         tc.tile_pool(name="osb", bufs=1) as osb, \
         tc.tile_pool(name="stats", bufs=4) as stats, \
         tc.tile_pool(name="psS", bufs=3, space="PSUM") as psS, \
         tc.tile_pool(name="psAV", bufs=1, space="PSUM") as psAV, \
         tc.tile_pool(name="psWO", bufs=1, space="PSUM") as psWO:
        wo_t = wop.tile([128, HL, D], BF16, tag="wo")
        nc.sync.dma_start(wo_t[:], wo4.rearrange("(ht p) m -> p ht m", p=128))
        if variant == "causal":
            dm_t = maskpl.tile([128, 4, 512], BF16, tag="dmask")
            nc.sync.dma_start(
                dm_t[:], T["dmask"].rearrange("p (j m) -> p j m", j=4))
        for qb in range(4):
            nch = (qb + 1) if variant == "causal" else 4  # key chunks of 512
            wq = nch * 512
            if variant == "general":
                mts = []
                for qt in range(4):
                    mt = maskpl.tile([128, S], BF16, tag=f"mask{qt}",
                                     name=f"mk{qb}_{qt}")
                    nc.sync.dma_start(mt[:], T["maskp"][ts(qb * 4 + qt, 128), :])
                    mts.append(mt)
            att = []
            for h in range(HL):
                pT_t = pTp.tile([128, NQT, 512], BF16, tag="pT", name=f"pT{qb}_{h}")
                for qt in range(4):
                    qsl = slice((qb * 4 + qt) * 128, (qb * 4 + qt + 1) * 128)
                    nph = (nch + 1) // 2
                    ph = [psS.tile([128, 1024], F32, tag="qk",
                                   name=f"qk{qb}_{h}_{qt}_{i}") for i in range(nph)]
                    off = (h % 2) * 64
                    for c in range(nch):
                        pp = ph[c // 2][:, (c % 2) * 512:(c % 2) * 512 + 512]
                        ksl = slice(c * 512, (c + 1) * 512)
                        nc.tensor.matmul(pp, qT[h][:, qsl], kT[h][:, ksl],
                                         start=True, stop=False)
                        if variant == "zero":
                            nc.tensor.matmul(pp, qrope[h // 2][off:off + 64, qsl],
                                             krope2[off:off + 64, ksl],
                                             start=False, stop=True)
                        elif variant == "causal":
                            isdiag = (c == qb)
                            nc.tensor.matmul(pp, qrope[h // 2][off:off + 64, qsl],
                                             krope2[off:off + 64, ksl],
                                             start=False, stop=not isdiag)
                            if isdiag:
                                nc.tensor.matmul(pp, ident_t[:], dm_t[:, qt, :],
                                                 start=False, stop=True)
                        else:
                            nc.tensor.matmul(pp, qrope[h // 2][off:off + 64, qsl],
                                             krope2[off:off + 64, ksl],
                                             start=False, stop=False)
                            nc.tensor.matmul(pp, ident_t[:], mts[qt][:, ksl],
                                             start=False, stop=True)
                    # widths actually written in each PSUM pair-tile
                    pw = [1024] * (nch // 2) + ([512] if nch % 2 else [])
                    mxs = []
                    for i in range(nph):
                        mx = stats.tile([128, 1], F32, tag=f"mx{i}",
                                        name=f"mx{i}_{qb}{h}{qt}")
                        nc.vector.reduce_max(mx[:], ph[i][:, 0:pw[i]], axis=AX.X)
                        mxs.append(mx)
                    if nph == 2:
                        mxc = stats.tile([128, 1], F32, tag="mxc",
                                         name=f"mxc{qb}{h}{qt}")
                        nc.vector.tensor_max(mxc[:], mxs[0][:], mxs[1][:])
                    else:
                        mxc = mxs[0]
                    negm = stats.tile([128, 1], F32, tag="negm", name=f"ng{qb}{h}{qt}")
                    nc.vector.tensor_scalar_mul(negm[:], mxc[:], -SCALE)
                    pu = pup.tile([128, S], BF16, tag="pu", name=f"pu{qb}{h}{qt}")
                    lts = []
                    for i in range(nph):
                        la = stats.tile([128, 1], F32, tag=f"l{i}",
                                        name=f"l{i}_{qb}{h}{qt}")
                        nc.scalar.activation(
                            pu[:, i * 1024:i * 1024 + pw[i]], ph[i][:, 0:pw[i]],
                            AF.Exp, bias=negm[:], scale=SCALE, accum_out=la[:])
                        lts.append(la)
                    if nph == 2:
                        lt = stats.tile([128, 1], F32, tag="lt", name=f"lt{qb}{h}{qt}")
                        nc.vector.tensor_add(lt[:], lts[0][:], lts[1][:])
                    else:
                        lt = lts[0]
                    rl = stats.tile([128, 1], F32, tag="rl", name=f"rl{qb}{h}{qt}")
                    nc.vector.reciprocal(rl[:], lt[:])
                    pn = pnp.tile([128, S], BF16, tag="pn", name=f"pn{qb}{h}{qt}")
                    nc.vector.tensor_scalar_mul(pn[:, 0:wq], pu[:, 0:wq], rl[:])
                    if taps and qb == 3 and h == 0:
                        nc.sync.dma_start(T["tap_p"][ts(qt, 128), 0:wq], pn[:, 0:wq])
                    nc.sync.dma_start(pT_t[:, 0:4 * nch, ts(qt, 128)], pn[:, 0:wq],
                                      transpose=True)
                pav = psAV.tile([128, 512], F32, tag="av", name=f"av{qb}_{h}")
                for kt in range(4 * nch):
                    nc.tensor.matmul(pav[:], v_all[:, kt, ts(h, 128)],
                                     pT_t[:, kt, :],
                                     start=(kt == 0), stop=(kt == 4 * nch - 1))
                at = attp.tile([128, 512], BF16, tag=f"att{h}", name=f"at{qb}_{h}")
                nc.scalar.activation(at[:], pav[:], AF.Copy)
                att.append(at)
            for qt in range(4):
                qrow = (qb * 4 + qt) * 128
                for dch in range(4):
                    pw2 = psWO.tile([128, 512], F32, tag="wops",
                                    name=f"wo{qb}{qt}{dch}")
                    for h in range(HL):
                        nc.tensor.matmul(pw2[:], att[h][:, ts(qt, 128)],
                                         wo_t[:, h, ts(dch, 512)],
                                         start=(h == 0), stop=(h == HL - 1))
                    ot = osb.tile([128, 512], F32, tag="ot", name=f"ot{qb}{qt}{dch}")
                    nc.scalar.activation(ot[:], pw2[:], AF.Copy)
                    nc.sync.dma_start(outp[qrow:qrow + 128, ts(dch, 512)], ot[:])

    kside_p.__exit__(None, None, None)
    qside_p.__exit__(None, None, None)
    rows_p.__exit__(None, None, None)
    kfeat_p.__exit__(None, None, None)
    dram_p.__exit__(None, None, None)
    const_p.__exit__(None, None, None)


def _classify_mask(mask):
    """mask: [S, S] float32. zero -> no mask; causal -> tril(0)/triu(very
    negative); general -> anything else."""
    if not mask.any():
        return "zero"
    tril = np.tril(np.ones((S, S), dtype=bool))
    if np.all(mask[tril] == 0.0) and np.all(mask[~tril] <= -1e8):
        return "causal"
    return "general"


def _shard(inputs, variant):
    x = np.asarray(inputs["x"], np.float32)
    mask = np.asarray(inputs["mask"], np.float32)[0, 0]
    pos_cos = np.asarray(inputs["pos_cos"], np.float32)
    pos_sin = np.asarray(inputs["pos_sin"], np.float32)
    W_DQ = np.asarray(inputs["W_DQ"], np.float32)
    W_UQ = np.asarray(inputs["W_UQ"], np.float32)
    W_UQR = np.asarray(inputs["W_UQR"], np.float32)
    W_DKV = np.asarray(inputs["W_DKV"], np.float32)
    W_UK = np.asarray(inputs["W_UK"], np.float32)
    W_UV = np.asarray(inputs["W_UV"], np.float32)
    W_DKR = np.asarray(inputs["W_DKR"], np.float32)
    W_O = np.asarray(inputs["W_O"], np.float32)
    qw = np.asarray(inputs["q_norm_w"], np.float32)
    kvw = np.asarray(inputs["kv_norm_w"], np.float32)

    cos4 = np.tile(np.ascontiguousarray(pos_cos.T), (4, 1)).astype(np.float32)
    sin4 = np.tile(np.ascontiguousarray(pos_sin.T), (4, 1)).astype(np.float32)
    wdkr = np.ascontiguousarray(
        np.concatenate([W_DKR[:, 0::2], W_DKR[:, 1::2]], axis=1))
    wuq_n = W_UQ * qw[:, None]
    wuqr_n = (W_UQR * qw[:, None]).reshape(DCQ, H, ROPE)
    wuk_n = W_UK * kvw[:, None]
    wuv_n = W_UV * kvw[:, None]
    ident = np.eye(128, dtype=np.float32).astype(BF)
    ones_r = np.ones((1, 128), np.float32)
    ones_c = np.ones((128, 1), np.float32)

    common = {}
    if variant == "general":
        common["maskp"] = (mask / SCALE).astype(BF)
    elif variant == "causal":
        # 4 diagonal-block triangle patterns: for q-tile j (within its
        # 512-key chunk), row r valid through key j*128 + r.
        r = np.arange(128)[:, None]
        kk = np.arange(512)[None, :]
        pats = [np.where(kk <= j * 128 + r, 0.0, -1e9 / SCALE) for j in range(4)]
        common["dmask"] = np.concatenate(pats, axis=1).astype(BF)

    in_maps = []
    for c in range(NCORES):
        b, g = divmod(c, 4)
        hs = slice(g * HL * NOPE, (g + 1) * HL * NOPE)
        heads = list(range(g * HL, (g + 1) * HL))
        wuqre = np.concatenate([wuqr_n[:, h, 0::2] for h in heads], axis=1)
        wuqro = np.concatenate([wuqr_n[:, h, 1::2] for h in heads], axis=1)
        in_maps.append({
            "xT": np.ascontiguousarray(x[b].T),
            "cos4": cos4,
            "sin4": sin4,
            "wdq": W_DQ,
            "wdkv": W_DKV,
            "wdkr": wdkr,
            "wuq": np.ascontiguousarray(wuq_n[:, hs]),
            "wuqre": np.ascontiguousarray(wuqre),
            "wuqro": np.ascontiguousarray(wuqro),
            "wuk": np.ascontiguousarray(wuk_n[:, hs]),
            "wuv": np.ascontiguousarray(wuv_n[:, hs]),
            "wo4": np.ascontiguousarray(W_O[hs, :]).astype(BF),
            "ident": ident,
            "ones_r": ones_r,
            "ones_c": ones_c,
            **common,
        })
    return in_maps


def kernel(**inputs):
    from concourse.bass_utils import run_bass_kernel_spmd

    variant = _classify_mask(np.asarray(inputs["mask"], np.float32)[0, 0])
    key = ("nc", variant)
    if key not in _BUILD_CACHE:
        _BUILD_CACHE[key] = build_nc(variant)
    nc = _BUILD_CACHE[key]
    _BUILD_CACHE["nc"] = nc  # latest build, for external inspection
    in_maps = _shard(inputs, variant)
    res = run_bass_kernel_spmd(nc, in_maps, core_ids=list(range(NCORES)))
    out = np.zeros((B, S, D), np.float32)
    for c in range(NCORES):
        out[c // 4] += np.asarray(res.results[c]["outp"], np.float32)
    return out


# revision 21
# speedup vs baseline: 1.0283x; 1.0283x over previous
"""MLA (DeepSeek-style) attention layer on 8 Trainium2 NeuronCores.

Sharding: core c -> batch b = c//4, head group g = c%4 (4 of 16 heads).
Each core computes a partial output (its heads' contribution through its
W_O row-slice); the host sums the 4 partials per batch.

Device layouts are feature-major ("transposed"): activations stored as
[feature, seq] so the PE contraction dim (partitions) is the feature dim.
The q/k path runs in fp32r (full-rate, ~1.6e-4 matmul precision) to keep
the attention logits accurate; probabilities/values/W_O run in bf16.
The probability matrix is transposed back via DMA-xbar (bf16) for the AV
matmul, and the final W_O matmul un-transposes into natural [seq, D].
RMSNorm weights are folded into the up-projection weights on the host;
the inverse-rms per-token scale is applied on PSUM eviction.

The kernel is mask-adaptive: the host classifies the mask input as
"zero" (all zeros -> dense attention, no mask applied), "causal"
(tril zeros / upper <= -1e8 -> block-causal skipping: QK chunks, softmax
stats, exp, transposes and AV tiles above the diagonal are skipped, and
the diagonal 512-key chunk gets one of 4 precomputed 128x512 triangle
patterns), or "general" (full additive-mask path). One program per
variant is built and cached.
"""
import sys

for _p in ("/opt/trn_rl_repo", "/root/.axon_site/_ro/trn_rl_repo"):
    if _p not in sys.path:
        sys.path.append(_p)

import numpy as np
import ml_dtypes

B, S, D = 2, 2048, 2048
H, NOPE, ROPE, VD = 16, 128, 64, 128
DCQ, DCKV = 1536, 512
EPS = 1e-6
SCALE = float(np.sqrt(NOPE + ROPE))
HL = 4           # local heads per core
NCORES = 8
NQT = S // 128   # 16
NKC = S // 512   # 4
NKD = D // 128   # 16
NMQ = DCQ // 128  # 12
NMKV = DCKV // 128  # 4
BF = ml_dtypes.bfloat16

_BUILD_CACHE = {}


def build_nc(variant="causal", taps=False):
    import concourse.tile as tile
    import concourse.mybir as mybir
    from concourse import bacc

    F32 = mybir.dt.float32
    F32R = mybir.dt.float32r
    BF16 = mybir.dt.bfloat16

    nc = bacc.Bacc(num_devices=NCORES)

    T = {}
    T["xT"] = nc.dram_tensor("xT", [D, S], F32R, kind="ExternalInput")
    if variant == "general":
        T["maskp"] = nc.dram_tensor("maskp", [S, S], BF16, kind="ExternalInput")
    elif variant == "causal":
        T["dmask"] = nc.dram_tensor("dmask", [128, 2048], BF16, kind="ExternalInput")
    T["cos4"] = nc.dram_tensor("cos4", [128, S], F32, kind="ExternalInput")
    T["sin4"] = nc.dram_tensor("sin4", [128, S], F32, kind="ExternalInput")
    T["wdq"] = nc.dram_tensor("wdq", [D, DCQ], F32R, kind="ExternalInput")
    T["wdkv"] = nc.dram_tensor("wdkv", [D, DCKV], F32R, kind="ExternalInput")
    T["wdkr"] = nc.dram_tensor("wdkr", [D, 64], F32R, kind="ExternalInput")
    T["wuq"] = nc.dram_tensor("wuq", [DCQ, HL * NOPE], F32R, kind="ExternalInput")
    T["wuqre"] = nc.dram_tensor("wuqre", [DCQ, HL * 32], F32R, kind="ExternalInput")
    T["wuqro"] = nc.dram_tensor("wuqro", [DCQ, HL * 32], F32R, kind="ExternalInput")
    T["wuk"] = nc.dram_tensor("wuk", [DCKV, HL * NOPE], F32R, kind="ExternalInput")
    T["wuv"] = nc.dram_tensor("wuv", [DCKV, HL * VD], F32R, kind="ExternalInput")
    T["wo4"] = nc.dram_tensor("wo4", [HL * VD, D], BF16, kind="ExternalInput")
    T["ident"] = nc.dram_tensor("ident", [128, 128], BF16, kind="ExternalInput")
    T["ones_r"] = nc.dram_tensor("ones_r", [1, 128], F32R, kind="ExternalInput")
    T["ones_c"] = nc.dram_tensor("ones_c", [128, 1], F32R, kind="ExternalInput")
    T["outp"] = nc.dram_tensor("outp", [S, D], F32, kind="ExternalOutput")
    if taps:
        T["tap_cq"] = nc.dram_tensor("tap_cq", [DCQ, S], F32R, kind="ExternalOutput")
        T["tap_inv"] = nc.dram_tensor("tap_inv", [2, S], F32, kind="ExternalOutput")
        T["tap_q"] = nc.dram_tensor("tap_q", [HL * NOPE, S], F32R, kind="ExternalOutput")
        T["tap_k"] = nc.dram_tensor("tap_k", [HL * NOPE, S], F32R, kind="ExternalOutput")
        T["tap_qr"] = nc.dram_tensor("tap_qr", [256, S], F32R, kind="ExternalOutput")
        T["tap_kr"] = nc.dram_tensor("tap_kr", [128, S], F32R, kind="ExternalOutput")
        T["tap_v"] = nc.dram_tensor("tap_v", [NQT * 128, HL * VD], BF16, kind="ExternalOutput")
        T["tap_p"] = nc.dram_tensor("tap_p", [512, S], BF16, kind="ExternalOutput")

    with tile.TileContext(nc) as tc:
        _emit(nc, tc, T, variant, taps)
    nc.compile()
    return nc


def _emit(nc, tc, T, variant, taps):
    import concourse.bass as bass
    import concourse.mybir as mybir

    F32 = mybir.dt.float32
    F32R = mybir.dt.float32r
    BF16 = mybir.dt.bfloat16
    AF = mybir.ActivationFunctionType
    AX = mybir.AxisListType
    ts = bass.ts

    xT, cos4, sin4 = T["xT"], T["cos4"], T["sin4"]
    wdq, wdkv, wdkr = T["wdq"], T["wdkv"], T["wdkr"]
    wuq, wuqre, wuqro, wuk, wuv, wo4 = (
        T["wuq"], T["wuqre"], T["wuqro"], T["wuk"], T["wuv"], T["wo4"])
    ident, ones_r, ones_c, outp = T["ident"], T["ones_r"], T["ones_c"], T["outp"]

    # --- persistent-scope pools, opened in lifetime (LIFO) order ---
    const_p = tc.tile_pool(name="constp", bufs=1)
    const = const_p.__enter__()
    onesr_t = const.tile([1, 128], F32R, tag="onesr")
    nc.sync.dma_start(onesr_t[:], ones_r[:])
    onesc_t = const.tile([128, 1], F32R, tag="onesc")
    nc.sync.dma_start(onesc_t[:], ones_c[:])
    ident_t = const.tile([128, 128], BF16, tag="ident")
    nc.sync.dma_start(ident_t[:], ident[:])

    dram_p = tc.tile_pool(name="dram", bufs=1, space="DRAM")
    dram = dram_p.__enter__()
    cqd = dram.tile([NMQ, 128, S], F32R, tag="cqd")
    ckvd = dram.tile([NMKV, 128, S], F32R, tag="ckvd")

    kfeat_p = tc.tile_pool(name="kfeat", bufs=1)
    kfeat = kfeat_p.__enter__()
    krope2 = kfeat.tile([128, S], F32R, tag="krope2")

    rows_p = tc.tile_pool(name="rowsp", bufs=1)
    rows = rows_p.__enter__()
    sum_rows = rows.tile([33, S], F32, tag="sum_rows")
    invq_row = rows.tile([1, S], F32R, tag="invq_row")
    invkv_row = rows.tile([1, S], F32R, tag="invkv_row")

    # ============ Phase A: down-projection (c spilled to DRAM) + k-rope ====
    # W_DQ (12 MiB) + W_DKR stay resident across all S-quarters; W_DKV
    # streams per quarter (SBUF won't hold all three plus x).
    with tc.tile_pool(name="wdown", bufs=1) as wdp, \
         tc.tile_pool(name="wkvA", bufs=2) as wkvp, \
         tc.tile_pool(name="xA", bufs=1) as xpool, \
         tc.tile_pool(name="evA", bufs=2) as evpool, \
         tc.tile_pool(name="sqA", bufs=2) as sqpool, \
         tc.tile_pool(name="ropeA", bufs=1) as ropeA, \
         tc.tile_pool(name="psA", bufs=3, space="PSUM") as psA, \
         tc.tile_pool(name="psSum", bufs=2, space="PSUM") as psSum:
        def _load_x(quar):
            sl = slice(quar * 512, (quar + 1) * 512)
            xq = []
            for k in range(NKD):
                t = xpool.tile([128, 512], F32R, tag=f"xq{k}", name=f"xq{quar}_{k}")
                nc.sync.dma_start(t[:], xT[ts(k, 128), sl])
                xq.append(t)
            return xq

        wd_tiles = []
        for m in range(NMQ):
            t = wdp.tile([128, NKD, 128], F32R, tag=f"wd{m}")
            wd_tiles.append(t)
        nc.sync.dma_start(
            wd_tiles[0][:],
            wdq.rearrange("(kt p) m -> p kt m", p=128)[:, :, 0:128])
        xq0 = _load_x(0)
        for m in range(1, NMQ):
            nc.sync.dma_start(
                wd_tiles[m][:],
                wdq.rearrange("(kt p) m -> p kt m", p=128)[:, :, m * 128:(m + 1) * 128])
        wkr_t = wdp.tile([128, NKD, 64], F32R, tag="wkr")
        nc.sync.dma_start(wkr_t[:], wdkr.rearrange("(kt p) m -> p kt m", p=128))
        for quar in range(4):
            sl = slice(quar * 512, (quar + 1) * 512)
            xq = xq0 if quar == 0 else _load_x(quar)
            sum_ps_q = psSum.tile([1, 512], F32, tag="sumq")
            sum_ps_kv = psSum.tile([1, 512], F32, tag="sumkv")
            for m in range(NMQ + NMKV):
                if m < NMQ:
                    wm = wd_tiles[m]
                else:
                    wm = wkvp.tile([128, NKD, 128], F32R, tag="wkv",
                                   name=f"wkv{quar}_{m - NMQ}")
                    nc.sync.dma_start(
                        wm[:],
                        wdkv.rearrange("(kt p) m -> p kt m", p=128)
                        [:, :, (m - NMQ) * 128:(m - NMQ + 1) * 128])
                ps = psA.tile([128, 512], F32, tag="dp", name=f"dp{quar}_{m}")
                for k in range(NKD):
                    nc.tensor.matmul(ps[:], wm[:, k, :], xq[k][:],
                                     start=(k == 0), stop=(k == NKD - 1))
                ev = evpool.tile([128, 512], F32R, tag="cev", name=f"cev{quar}_{m}")
                nc.scalar.activation(ev[:], ps[:], AF.Copy)
                dst = cqd[m] if m < NMQ else ckvd[m - NMQ]
                nc.sync.dma_start(dst[:, sl], ev[:])
                sq = sqpool.tile([128, 512], F32R, tag="sq", name=f"sq{quar}_{m}")
                nc.scalar.activation(sq[:], ps[:], AF.Square)
                if m < NMQ:
                    nc.tensor.matmul(sum_ps_q[:], onesc_t[:], sq[:],
                                     start=(m == 0), stop=(m == NMQ - 1))
                else:
                    nc.tensor.matmul(sum_ps_kv[:], onesc_t[:], sq[:],
                                     start=(m == NMQ), stop=(m == NMQ + NMKV - 1))
            # merged 64-col W_DKR tile (even rope dims in 0:32, odd in 32:64)
            psr = psA.tile([128, 512], F32, tag="dp", name=f"dpr{quar}")
            for k in range(NKD):
                nc.tensor.matmul(psr[:64, :], wkr_t[:, k, :], xq[k][:],
                                 start=(k == 0), stop=(k == NKD - 1))
            kre_q = ropeA.tile([32, 512], F32, tag="kr0", name=f"kre{quar}")
            nc.scalar.activation(kre_q[:], psr[0:32, :], AF.Copy)
            kro_q = ropeA.tile([32, 512], F32, tag="kr1", name=f"kro{quar}")
            nc.scalar.activation(kro_q[:], psr[32:64, :], AF.Copy)
            # k-side rope for this quarter (k_R has no rms norm)
            cs_a = ropeA.tile([32, 512], F32, tag="cs_a", name=f"cs{quar}")
            nc.sync.dma_start(cs_a[:], cos4[0:32, sl])
            sn_a = ropeA.tile([32, 512], F32, tag="sn_a", name=f"sn{quar}")
            nc.sync.dma_start(sn_a[:], sin4[0:32, sl])
            t1k = ropeA.tile([32, 512], F32, tag="t1k", bufs=1, name=f"t1k{quar}")
            nc.vector.tensor_mul(t1k[:], kre_q[:], cs_a[:])
            t2k = ropeA.tile([32, 512], F32, tag="t2k", bufs=1, name=f"t2k{quar}")
            nc.vector.tensor_mul(t2k[:], kro_q[:], sn_a[:])
            ko1 = ropeA.tile([32, 512], F32R, tag="ko1", name=f"ko1{quar}")
            nc.vector.tensor_sub(ko1[:], t1k[:], t2k[:])
            t3k = ropeA.tile([32, 512], F32, tag="t1k", bufs=1, name=f"t3k{quar}")
            nc.vector.tensor_mul(t3k[:], kre_q[:], sn_a[:])
            t4k = ropeA.tile([32, 512], F32, tag="t2k", bufs=1, name=f"t4k{quar}")
            nc.vector.tensor_mul(t4k[:], kro_q[:], cs_a[:])
            ko2 = ropeA.tile([32, 512], F32R, tag="ko2", name=f"ko2{quar}")
            nc.vector.tensor_add(ko2[:], t3k[:], t4k[:])
            for rep in range(2):
                nc.sync.dma_start(krope2[ts(rep * 2, 32), sl], ko1[:])
                nc.sync.dma_start(krope2[ts(rep * 2 + 1, 32), sl], ko2[:])
            nc.vector.tensor_copy(sum_rows[0:1, sl], sum_ps_q[:])
            nc.vector.tensor_copy(sum_rows[32:33, sl], sum_ps_kv[:])

    # ---- inverse rms rows (chunk broadcasts built on demand in B1/B2) ----
    with tc.tile_pool(name="invp", bufs=1) as invp:
        epst = invp.tile([1, 1], F32, tag="epst")
        nc.gpsimd.memset(epst[:], EPS)
        rms2 = invp.tile([33, S], F32, tag="rms2")
        nc.scalar.activation(rms2[0:1, :], sum_rows[0:1, :], AF.Sqrt,
                             bias=epst[:], scale=1.0 / DCQ)
        nc.scalar.activation(rms2[32:33, :], sum_rows[32:33, :], AF.Sqrt,
                             bias=epst[:], scale=1.0 / DCKV)
        with nc.allow_low_precision(reason="f32r shares f32 bits"):
            nc.vector.reciprocal(invq_row[:], rms2[0:1, :])
            nc.vector.reciprocal(invkv_row[:], rms2[32:33, :])
    if taps:
        nc.sync.dma_start(T["tap_inv"][0:1, :], invq_row[:])
        nc.sync.dma_start(T["tap_inv"][1:2, :], invkv_row[:])
        for m in range(NMQ):
            nc.sync.dma_start(T["tap_cq"][ts(m, 128), :], cqd[m])

    # ============ Phase B1: q-side up-projection + rope ============
    qside_p = tc.tile_pool(name="qside", bufs=1)
    qside = qside_p.__enter__()
    qT = [qside.tile([128, S], F32R, tag=f"qT{h}", name=f"qT{h}") for h in range(HL)]
    qrope = [qside.tile([128, S], F32R, tag=f"qrope{p}", name=f"qrope{p}")
             for p in range(2)]

    with tc.tile_pool(name="wB1", bufs=1) as wb1, \
         tc.tile_pool(name="csB1", bufs=2) as csB1, \
         tc.tile_pool(name="cqs", bufs=2) as cqs, \
         tc.tile_pool(name="bcB1", bufs=2) as bcB1, \
         tc.tile_pool(name="ropeS", bufs=2) as ropeS, \
         tc.tile_pool(name="psB", bufs=3, space="PSUM") as psB:
        wuq_t = wb1.tile([128, NMQ, HL * NOPE], F32R, tag="wuq")
        nc.sync.dma_start(wuq_t[:], wuq.rearrange("(kt p) m -> p kt m", p=128))
        wuqre_t = wb1.tile([128, NMQ, HL * 32], F32R, tag="wuqre")
        nc.sync.dma_start(wuqre_t[:], wuqre.rearrange("(kt p) m -> p kt m", p=128))
        wuqro_t = wb1.tile([128, NMQ, HL * 32], F32R, tag="wuqro")
        nc.sync.dma_start(wuqro_t[:], wuqro.rearrange("(kt p) m -> p kt m", p=128))
        for n in range(NKC):
            sl = slice(n * 512, (n + 1) * 512)
            cos_t = csB1.tile([128, 512], F32, tag="cos", name=f"cos{n}")
            nc.sync.dma_start(cos_t[:], cos4[:, sl])
            sin_t = csB1.tile([128, 512], F32, tag="sin", name=f"sin{n}")
            nc.sync.dma_start(sin_t[:], sin4[:, sl])
            cq = []
            for k in range(NMQ):
                t = cqs.tile([128, 512], F32R, tag=f"cqc{k}", name=f"cqc{n}_{k}")
                nc.sync.dma_start(t[:], cqd[k][:, sl])
                cq.append(t)
            psbc = psB.tile([128, 512], F32, tag="bc", bufs=2, name=f"bcq{n}")
            nc.tensor.matmul(psbc[:], onesr_t[:], invq_row[0:1, sl],
                             start=True, stop=True)
            bcq_t = bcB1.tile([128, 512], F32, tag="bcq", name=f"bcqs{n}")
            nc.scalar.activation(bcq_t[:], psbc[:], AF.Copy)
            for h in range(HL):
                ps = psB.tile([128, 512], F32, tag="up", name=f"upq{n}_{h}")
                for k in range(NMQ):
                    nc.tensor.matmul(ps[:], wuq_t[:, k, ts(h, 128)], cq[k][:],
                                     start=(k == 0), stop=(k == NMQ - 1))
                nc.vector.tensor_mul(qT[h][:, sl], ps[:], bcq_t[:])
            psE = psB.tile([128, 512], F32, tag="up", name=f"upe{n}")
            for k in range(NMQ):
                nc.tensor.matmul(psE[:], wuqre_t[:, k, :], cq[k][:],
                                 start=(k == 0), stop=(k == NMQ - 1))
            esc = ropeS.tile([128, 512], F32, tag="esc", bufs=1, name=f"esc{n}")
            nc.vector.tensor_mul(esc[:], psE[:], bcq_t[:])
            psO = psB.tile([128, 512], F32, tag="up", name=f"upo{n}")
            for k in range(NMQ):
                nc.tensor.matmul(psO[:], wuqro_t[:, k, :], cq[k][:],
                                 start=(k == 0), stop=(k == NMQ - 1))
            osc = ropeS.tile([128, 512], F32, tag="osc", bufs=1, name=f"osc{n}")
            nc.vector.tensor_mul(osc[:], psO[:], bcq_t[:])
            t1 = ropeS.tile([128, 512], F32, tag="t1", bufs=1, name=f"t1{n}")
            nc.vector.tensor_mul(t1[:], esc[:], cos_t[:])
            t2 = ropeS.tile([128, 512], F32, tag="t2", bufs=1, name=f"t2{n}")
            nc.vector.tensor_mul(t2[:], osc[:], sin_t[:])
            o1 = ropeS.tile([128, 512], F32R, tag="o1", name=f"o1{n}")
            nc.vector.tensor_sub(o1[:], t1[:], t2[:])
            t3 = ropeS.tile([128, 512], F32, tag="t1", bufs=1, name=f"t3{n}")
            nc.vector.tensor_mul(t3[:], esc[:], sin_t[:])
            t4 = ropeS.tile([128, 512], F32, tag="t2", bufs=1, name=f"t4{n}")
            nc.vector.tensor_mul(t4[:], osc[:], cos_t[:])
            o2 = ropeS.tile([128, 512], F32R, tag="o2", name=f"o2{n}")
            nc.vector.tensor_add(o2[:], t3[:], t4[:])
            for h in range(HL):
                p, off = h // 2, (h % 2) * 64
                nc.sync.dma_start(qrope[p][off:off + 32, sl], o1[ts(h, 32), :])
                nc.sync.dma_start(qrope[p][off + 32:off + 64, sl], o2[ts(h, 32), :])

    if taps:
        for h in range(HL):
            nc.sync.dma_start(T["tap_q"][ts(h, 128), :], qT[h][:])
        nc.sync.dma_start(T["tap_qr"][0:128, :], qrope[0][:])
        nc.sync.dma_start(T["tap_qr"][128:256, :], qrope[1][:])
        nc.sync.dma_start(T["tap_kr"][:], krope2[:])

    # ============ Phase B2: kv-side up-projection ============
    kside_p = tc.tile_pool(name="kside", bufs=1)
    kside = kside_p.__enter__()
    kT = [kside.tile([128, S], F32R, tag=f"kT{h}", name=f"kT{h}") for h in range(HL)]
    v_all = kside.tile([128, NQT, HL * VD], BF16, tag="v_all")

    with tc.tile_pool(name="wB2", bufs=1) as wb2, \
         tc.tile_pool(name="ckvs", bufs=2) as ckvs, \
         tc.tile_pool(name="psB2", bufs=3, space="PSUM") as psB2:
        wuk_t = wb2.tile([128, NMKV, HL * NOPE], F32R, tag="wuk")
        nc.sync.dma_start(wuk_t[:], wuk.rearrange("(kt p) m -> p kt m", p=128))
        wuv_t = wb2.tile([128, NMKV, HL * VD], F32R, tag="wuv")
        nc.sync.dma_start(wuv_t[:], wuv.rearrange("(kt p) m -> p kt m", p=128))
        for n in range(NKC):
            sl = slice(n * 512, (n + 1) * 512)
            psbc = psB2.tile([128, 512], F32, tag="bc", bufs=2, name=f"bckv{n}")
            nc.tensor.matmul(psbc[:], onesr_t[:], invkv_row[0:1, sl],
                             start=True, stop=True)
            bckv_t = ckvs.tile([128, 512], F32, tag="bckv", name=f"bckvs{n}")
            nc.scalar.activation(bckv_t[:], psbc[:], AF.Copy)
            ckv = []
            for k in range(NMKV):
                t = ckvs.tile([128, 512], F32R, tag=f"ckvc{k}", name=f"ckvc{n}_{k}")
                nc.sync.dma_start(t[:], ckvd[k][:, sl])
                tn = ckvs.tile([128, 512], F32R, tag=f"ckvn{k}", name=f"ckvn{n}_{k}")
                nc.vector.tensor_mul(tn[:], t[:], bckv_t[:])
                ckv.append(tn)
            for h in range(HL):
                ps = psB2.tile([128, 512], F32, tag="upk", name=f"upk{n}_{h}")
                for k in range(NMKV):
                    nc.tensor.matmul(ps[:], wuk_t[:, k, ts(h, 128)], ckv[k][:],
                                     start=(k == 0), stop=(k == NMKV - 1))
                nc.scalar.activation(kT[h][:, sl], ps[:], AF.Copy)
            for vm in range(4):
                m = n * 4 + vm
                ps = psB2.tile([128, 512], F32, tag="upv", name=f"upv{n}_{vm}")
                for k in range(NMKV):
                    nc.tensor.matmul(ps[:], ckv[k][:, ts(vm, 128)], wuv_t[:, k, :],
                                     start=(k == 0), stop=(k == NMKV - 1))
                nc.scalar.activation(v_all[:, m, :], ps[:], AF.Copy)
    if taps:
        for h in range(HL):
            nc.sync.dma_start(T["tap_k"][ts(h, 128), :], kT[h][:])
        for m in range(NQT):
            nc.sync.dma_start(T["tap_v"][ts(m, 128), :], v_all[:, m, :])

    # ============ Attention ============
    with tc.tile_pool(name="wo", bufs=1) as wop, \
         tc.tile_pool(name="maskP", bufs=1) as maskpl, \
         tc.tile_pool(name="pu", bufs=2) as pup, \
         tc.tile_pool(name="pn", bufs=2) as pnp, \
         tc.tile_pool(name="pT", bufs=2) as pTp, \
         tc.tile_pool(name="attP", bufs=1) as attp, \
         tc.tile_pool(name="osb", bufs=1) as osb, \
         tc.tile_pool(name="stats", bufs=4) as stats, \
         tc.tile_pool(name="psS", bufs=3, space="PSUM") as psS, \
         tc.tile_pool(name="psAV", bufs=1, space="PSUM") as psAV, \
         tc.tile_pool(name="psWO", bufs=1, space="PSUM") as psWO:
        wo_t = wop.tile([128, HL, D], BF16, tag="wo")
        nc.sync.dma_start(wo_t[:], wo4.rearrange("(ht p) m -> p ht m", p=128))
        if variant == "causal":
            dm_t = maskpl.tile([128, 4, 512], BF16, tag="dmask")
            nc.sync.dma_start(
                dm_t[:], T["dmask"].rearrange("p (j m) -> p j m", j=4))
        for qb in range(4):
            nch = (qb + 1) if variant == "causal" else 4  # key chunks of 512
            wq = nch * 512
            if variant == "general":
                mts = []
                for qt in range(4):
                    mt = maskpl.tile([128, S], BF16, tag=f"mask{qt}",
                                     name=f"mk{qb}_{qt}")
                    nc.sync.dma_start(mt[:], T["maskp"][ts(qb * 4 + qt, 128), :])
                    mts.append(mt)
            att = []
            for h in range(HL):
                pT_t = pTp.tile([128, NQT, 512], BF16, tag="pT", name=f"pT{qb}_{h}")
                for qt in range(4):
                    qsl = slice((qb * 4 + qt) * 128, (qb * 4 + qt + 1) * 128)
                    nph = (nch + 1) // 2
                    ph = [psS.tile([128, 1024], F32, tag="qk",
                                   name=f"qk{qb}_{h}_{qt}_{i}") for i in range(nph)]
                    off = (h % 2) * 64
                    for c in range(nch):
                        pp = ph[c // 2][:, (c % 2) * 512:(c % 2) * 512 + 512]
                        ksl = slice(c * 512, (c + 1) * 512)
                        nc.tensor.matmul(pp, qT[h][:, qsl], kT[h][:, ksl],
                                         start=True, stop=False)
                        if variant == "zero":
                            nc.tensor.matmul(pp, qrope[h // 2][off:off + 64, qsl],
                                             krope2[off:off + 64, ksl],
                                             start=False, stop=True)
                        elif variant == "causal":
                            isdiag = (c == qb)
                            nc.tensor.matmul(pp, qrope[h // 2][off:off + 64, qsl],
                                             krope2[off:off + 64, ksl],
                                             start=False, stop=not isdiag)
                            if isdiag:
                                nc.tensor.matmul(pp, ident_t[:], dm_t[:, qt, :],
                                                 start=False, stop=True)
                        else:
                            nc.tensor.matmul(pp, qrope[h // 2][off:off + 64, qsl],
                                             krope2[off:off + 64, ksl],
                                             start=False, stop=False)
                            nc.tensor.matmul(pp, ident_t[:], mts[qt][:, ksl],
                                             start=False, stop=True)
                    # widths actually written in each PSUM pair-tile
                    pw = [1024] * (nch // 2) + ([512] if nch % 2 else [])
                    mxs = []
                    for i in range(nph):
                        mx = stats.tile([128, 1], F32, tag=f"mx{i}",
                                        name=f"mx{i}_{qb}{h}{qt}")
                        nc.vector.reduce_max(mx[:], ph[i][:, 0:pw[i]], axis=AX.X)
                        mxs.append(mx)
                    if nph == 2:
                        mxc = stats.tile([128, 1], F32, tag="mxc",
                                         name=f"mxc{qb}{h}{qt}")
                        nc.vector.tensor_max(mxc[:], mxs[0][:], mxs[1][:])
                    else:
                        mxc = mxs[0]
                    negm = stats.tile([128, 1], F32, tag="negm", name=f"ng{qb}{h}{qt}")
                    nc.vector.tensor_scalar_mul(negm[:], mxc[:], -SCALE)
                    pu = pup.tile([128, S], BF16, tag="pu", name=f"pu{qb}{h}{qt}")
                    lts = []
                    for i in range(nph):
                        la = stats.tile([128, 1], F32, tag=f"l{i}",
                                        name=f"l{i}_{qb}{h}{qt}")
                        nc.scalar.activation(
                            pu[:, i * 1024:i * 1024 + pw[i]], ph[i][:, 0:pw[i]],
                            AF.Exp, bias=negm[:], scale=SCALE, accum_out=la[:])
                        lts.append(la)
                    if nph == 2:
                        lt = stats.tile([128, 1], F32, tag="lt", name=f"lt{qb}{h}{qt}")
                        nc.vector.tensor_add(lt[:], lts[0][:], lts[1][:])
                    else:
                        lt = lts[0]
                    rl = stats.tile([128, 1], F32, tag="rl", name=f"rl{qb}{h}{qt}")
                    nc.vector.reciprocal(rl[:], lt[:])
                    pn = pnp.tile([128, S], BF16, tag="pn", name=f"pn{qb}{h}{qt}")
                    nc.vector.tensor_scalar_mul(pn[:, 0:wq], pu[:, 0:wq], rl[:])
                    if taps and qb == 3 and h == 0:
                        nc.sync.dma_start(T["tap_p"][ts(qt, 128), 0:wq], pn[:, 0:wq])
                    nc.sync.dma_start(pT_t[:, 0:4 * nch, ts(qt, 128)], pn[:, 0:wq],
                                      transpose=True)
                pav = psAV.tile([128, 512], F32, tag="av", name=f"av{qb}_{h}")
                for kt in range(4 * nch):
                    nc.tensor.matmul(pav[:], v_all[:, kt, ts(h, 128)],
                                     pT_t[:, kt, :],
                                     start=(kt == 0), stop=(kt == 4 * nch - 1))
                at = attp.tile([128, 512], BF16, tag=f"att{h}", name=f"at{qb}_{h}")
                nc.scalar.activation(at[:], pav[:], AF.Copy)
                att.append(at)
            for qt in range(4):
                qrow = (qb * 4 + qt) * 128
                for dch in range(4):
                    pw2 = psWO.tile([128, 512], F32, tag="wops",
                                    name=f"wo{qb}{qt}{dch}")
                    for h in range(HL):
                        nc.tensor.matmul(pw2[:], att[h][:, ts(qt, 128)],
                                         wo_t[:, h, ts(dch, 512)],
                                         start=(h == 0), stop=(h == HL - 1))
                    ot = osb.tile([128, 512], F32, tag="ot", name=f"ot{qb}{qt}{dch}")
                    nc.scalar.activation(ot[:], pw2[:], AF.Copy)
                    nc.sync.dma_start(outp[qrow:qrow + 128, ts(dch, 512)], ot[:])

    kside_p.__exit__(None, None, None)
    qside_p.__exit__(None, None, None)
    rows_p.__exit__(None, None, None)
    kfeat_p.__exit__(None, None, None)
    dram_p.__exit__(None, None, None)
    const_p.__exit__(None, None, None)


def _classify_mask(mask):
    """mask: [S, S] float32. zero -> no mask; causal -> tril(0)/triu(very
    negative); general -> anything else."""
    if not mask.any():
        return "zero"
    tril = np.tril(np.ones((S, S), dtype=bool))
    if np.all(mask[tril] == 0.0) and np.all(mask[~tril] <= -1e8):
        return "causal"
    return "general"


def _shard(inputs, variant):
    x = np.asarray(inputs["x"], np.float32)
    mask = np.asarray(inputs["mask"], np.float32)[0, 0]
    pos_cos = np.asarray(inputs["pos_cos"], np.float32)
    pos_sin = np.asarray(inputs["pos_sin"], np.float32)
    W_DQ = np.asarray(inputs["W_DQ"], np.float32)
    W_UQ = np.asarray(inputs["W_UQ"], np.float32)
    W_UQR = np.asarray(inputs["W_UQR"], np.float32)
    W_DKV = np.asarray(inputs["W_DKV"], np.float32)
    W_UK = np.asarray(inputs["W_UK"], np.float32)
    W_UV = np.asarray(inputs["W_UV"], np.float32)
    W_DKR = np.asarray(inputs["W_DKR"], np.float32)
    W_O = np.asarray(inputs["W_O"], np.float32)
    qw = np.asarray(inputs["q_norm_w"], np.float32)
    kvw = np.asarray(inputs["kv_norm_w"], np.float32)

    cos4 = np.tile(np.ascontiguousarray(pos_cos.T), (4, 1)).astype(np.float32)
    sin4 = np.tile(np.ascontiguousarray(pos_sin.T), (4, 1)).astype(np.float32)
    wdkr = np.ascontiguousarray(
        np.concatenate([W_DKR[:, 0::2], W_DKR[:, 1::2]], axis=1))
    wuq_n = W_UQ * qw[:, None]
    wuqr_n = (W_UQR * qw[:, None]).reshape(DCQ, H, ROPE)
    wuk_n = W_UK * kvw[:, None]
    wuv_n = W_UV * kvw[:, None]
    ident = np.eye(128, dtype=np.float32).astype(BF)
    ones_r = np.ones((1, 128), np.float32)
    ones_c = np.ones((128, 1), np.float32)

    common = {}
    if variant == "general":
        common["maskp"] = (mask / SCALE).astype(BF)
    elif variant == "causal":
        # 4 diagonal-block triangle patterns: for q-tile j (within its
        # 512-key chunk), row r valid through key j*128 + r.
        r = np.arange(128)[:, None]
        kk = np.arange(512)[None, :]
        pats = [np.where(kk <= j * 128 + r, 0.0, -1e9 / SCALE) for j in range(4)]
        common["dmask"] = np.concatenate(pats, axis=1).astype(BF)

    in_maps = []
    for c in range(NCORES):
        b, g = divmod(c, 4)
        hs = slice(g * HL * NOPE, (g + 1) * HL * NOPE)
        heads = list(range(g * HL, (g + 1) * HL))
        wuqre = np.concatenate([wuqr_n[:, h, 0::2] for h in heads], axis=1)
        wuqro = np.concatenate([wuqr_n[:, h, 1::2] for h in heads], axis=1)
        in_maps.append({
            "xT": np.ascontiguousarray(x[b].T),
            "cos4": cos4,
            "sin4": sin4,
            "wdq": W_DQ,
            "wdkv": W_DKV,
            "wdkr": wdkr,
            "wuq": np.ascontiguousarray(wuq_n[:, hs]),
            "wuqre": np.ascontiguousarray(wuqre),
            "wuqro": np.ascontiguousarray(wuqro),
            "wuk": np.ascontiguousarray(wuk_n[:, hs]),
            "wuv": np.ascontiguousarray(wuv_n[:, hs]),
            "wo4": np.ascontiguousarray(W_O[hs, :]).astype(BF),
            "ident": ident,
            "ones_r": ones_r,
            "ones_c": ones_c,
            **common,
        })
    return in_maps


def kernel(**inputs):
    from concourse.bass_utils import run_bass_kernel_spmd

    variant = _classify_mask(np.asarray(inputs["mask"], np.float32)[0, 0])
    key = ("nc", variant)
    if key not in _BUILD_CACHE:
        _BUILD_CACHE[key] = build_nc(variant)
    nc = _BUILD_CACHE[key]
    _BUILD_CACHE["nc"] = nc  # latest build, for external inspection
    in_maps = _shard(inputs, variant)
    res = run_bass_kernel_spmd(nc, in_maps, core_ids=list(range(NCORES)))
    out = np.zeros((B, S, D), np.float32)
    for c in range(NCORES):
        out[c // 4] += np.asarray(res.results[c]["outp"], np.float32)
    return out
